# revision 1
# baseline (speedup 1.0000x reference)
"""Causal multi-head self-attention (B=8, S=2048, D=384, H=4, Hd=96) on 8
Trainium2 NeuronCores.

Sharding: data-parallel over batch — each core processes one batch element,
weights replicated. No collectives needed.

Per-core algorithm (flash-style, fully SBUF-resident, no attention matrix in
HBM):
  - host passes x[b] pre-transposed as xT [384, 2048] (layout prep only)
  - QT/KT computed per head in [96, S] layout (d on partitions)
  - V' = [V_h | ones] natural layout [S, 97*4] via augmented weight matrix
    (bias + ones column folded into the projection contraction)
  - scoresT[k, q] = KT_h^T @ QT_h per 128-row k-tile and 512-col q-chunk;
    exp on ScalarE (PSUM->SBUF, scale=1/sqrt(Hd) folded in); causal diagonal
    blocks zeroed post-exp by multiplying a 0/1 mask on GpSimd
  - OT' accumulated in PSUM: rows 0..95 = unnormalized head output (d x q),
    row 96 = softmax denominator (from the ones column of V')
  - reciprocal (custom DVE approx) -> partition_broadcast -> multiply
  - output projection per head directly from normalized [96, S] tiles,
    summed in PSUM across heads, bias via a rank-1 ones matmul, DMA to HBM
"""

import os
import sys

sys.path.insert(0, "/opt/trn_rl_repo")

import numpy as np

import concourse.bass as bass
import concourse.tile as tile
from concourse import bacc, mybir
from concourse.bass_utils import run_bass_kernel_spmd

N_CORES = 8
S = 2048
D = 384
H = 4
HD = 96
CH = 512          # q-chunk width (columns per matmul)
NCH = S // CH     # 4 q-chunks
P = 128           # k-tile height / partition dim
KTN = S // P      # 16 k-tiles
SCALE = 1.0 / np.sqrt(HD)

F32 = mybir.dt.float32
MM_DT = os.environ.get("ATTN_MM_DT", "float32r")  # float32r | float32


def _split_groups(n, g=3):
    """Split n k-tiles into exp groups of <=g (PSUM tile = g banks)."""
    out = []
    while n > 0:
        if g == 3 and n == 4:
            out += [2, 2]
            break
        take = min(g, n)
        out.append(take)
        n -= take
    return out


def build_nc(repeat=1, variant=(), loop_n=0):
    nc = bacc.Bacc("TRN2", target_bir_lowering=False, debug=False,
                   enable_asserts=False, num_devices=N_CORES)
    # MF: dtype for tensors feeding matmuls (float32r = single-pass relaxed
    # fp32 on the PE, 4x faster than true fp32; same 4-byte numpy layout)
    MF = mybir.dt.float32r if MM_DT == "float32r" else F32

    xt_d = nc.dram_tensor("xt", [D, S], MF, kind="ExternalInput").ap()
    wq_d = nc.dram_tensor("wq", [D, D], MF, kind="ExternalInput").ap()
    wk_d = nc.dram_tensor("wk", [D, D], MF, kind="ExternalInput").ap()
    wvx_d = nc.dram_tensor("wvx", [D + 1, 97 * H], MF, kind="ExternalInput").ap()
    wo_d = nc.dram_tensor("wo", [D, D], MF, kind="ExternalInput").ap()
    bqh_d = nc.dram_tensor("bqh", [HD, H], F32, kind="ExternalInput").ap()
    bkh_d = nc.dram_tensor("bkh", [HD, H], F32, kind="ExternalInput").ap()
    bo_d = nc.dram_tensor("bo", [1, D], MF, kind="ExternalInput").ap()
    msk_d = nc.dram_tensor("msk", [P, 4 * CH], MF, kind="ExternalInput").ap()
    ones_d = nc.dram_tensor("onesrow", [1, S], MF, kind="ExternalInput").ap()
    out_d = nc.dram_tensor("out", [S, D], F32, kind="ExternalOutput").ap()

    Exp = mybir.ActivationFunctionType.Exp
    mult = mybir.AluOpType.mult

    with tile.TileContext(nc) as tc:
        wpool = tc.alloc_tile_pool(name="w", bufs=1)
        xpool = tc.alloc_tile_pool(name="x", bufs=1)
        qkt_pool = tc.alloc_tile_pool(name="qkt", bufs=1)
        vpool = tc.alloc_tile_pool(name="v", bufs=1)
        ppool = tc.alloc_tile_pool(name="p", bufs=3 if "grp3" in variant else 4)
        onpool = tc.alloc_tile_pool(name="on", bufs=2)
        rpool = tc.alloc_tile_pool(name="r", bufs=3)
        GRP = 3 if "grp3" in variant else 2
        ACC_BUFS = 2 if "grp3" in variant else 4
        qkpool = tc.alloc_tile_pool(name="qkps", bufs=2, space="PSUM")
        accpool = tc.alloc_tile_pool(name="accps", bufs=ACC_BUFS, space="PSUM")

        import contextlib
        loop_ctx = (tc.For_i(0, loop_n, 1) if loop_n
                    else contextlib.nullcontext())
        with loop_ctx:
          for _rep in range(repeat):
              # ---- load weights / constants ----
              xt_sb, wq_sb, wk_sb, wv_sb, wo_sb = [], [], [], [], []
              for t in range(3):
                  xt = xpool.tile([P, S], MF, name=f"xt{t}", tag=f"xt{t}")
                  nc.sync.dma_start(xt[:], xt_d[P * t:P * t + P, :])
                  xt_sb.append(xt)
                  wqt = wpool.tile([P, D], MF, name=f"wq{t}", tag=f"wq{t}")
                  nc.sync.dma_start(wqt[:], wq_d[P * t:P * t + P, :])
                  wq_sb.append(wqt)
                  wkt = wpool.tile([P, D], MF, name=f"wk{t}", tag=f"wk{t}")
                  nc.sync.dma_start(wkt[:], wk_d[P * t:P * t + P, :])
                  wk_sb.append(wkt)
                  wvt = wpool.tile([P, 97 * H], MF, name=f"wv{t}", tag=f"wv{t}")
                  nc.sync.dma_start(wvt[:], wvx_d[P * t:P * t + P, :])
                  wv_sb.append(wvt)
              wvb = wpool.tile([1, 97 * H], MF, name="wvb", tag="wvb")
              nc.sync.dma_start(wvb[:], wvx_d[D:D + 1, :])
              for h in range(H):
                  wot = wpool.tile([HD, D], MF, name=f"wo{h}", tag=f"wo{h}")
                  nc.sync.dma_start(wot[:], wo_d[HD * h:HD * h + HD, :])
                  wo_sb.append(wot)
              bq_sb = wpool.tile([HD, H], F32, name="bq", tag="bq")
              nc.sync.dma_start(bq_sb[:], bqh_d[:, :])
              bk_sb = wpool.tile([HD, H], F32, name="bk", tag="bk")
              nc.sync.dma_start(bk_sb[:], bkh_d[:, :])
              bo_sb = wpool.tile([1, D], MF, name="bo", tag="bo")
              nc.sync.dma_start(bo_sb[:], bo_d[:, :])
              msk_sb = wpool.tile([P, 4 * CH], MF, name="msk", tag="msk")
              nc.sync.dma_start(msk_sb[:], msk_d[:, :])
              ones = wpool.tile([1, S], MF, name="ones", tag="ones")
              nc.sync.dma_start(ones[:], ones_d[:, :])

              # ---- Q/K projections: per-head transposed layout [96, S] ----
              qt_sb, kt_sb = [], []
              for h in range(H):
                  qt = qkt_pool.tile([HD, S], MF, name=f"qt{h}", tag=f"qt{h}")
                  qt_sb.append(qt)
                  kt = qkt_pool.tile([HD, S], MF, name=f"kt{h}", tag=f"kt{h}")
                  kt_sb.append(kt)
              for w_sb, b_sb, dst in ((wq_sb, bq_sb, qt_sb), (wk_sb, bk_sb, kt_sb)):
                  for h in range(H):
                      for ci in range(NCH):
                          ps = accpool.tile([HD, CH], F32, name="projps", tag="acc")
                          for t in range(3):
                              nc.tensor.matmul(
                                  ps[:],
                                  w_sb[t][:, HD * h:HD * h + HD],
                                  xt_sb[t][:, CH * ci:CH * ci + CH],
                                  start=(t == 0), stop=(t == 2))
                          nc.scalar.add(
                              dst[h][:, CH * ci:CH * ci + CH], ps[:], b_sb[:, h:h + 1])

              # ---- V' projection: natural layout [S, 97*H] with ones column ----
              v_sb = []
              for st in range(KTN):
                  ps = accpool.tile([P, 97 * H], F32, name="vps", tag="acc")
                  for t in range(3):
                      nc.tensor.matmul(ps[:], xt_sb[t][:, P * st:P * st + P],
                                       wv_sb[t][:], start=(t == 0), stop=False)
                  nc.tensor.matmul(ps[:], ones[:, 0:P], wvb[:],
                                   start=False, stop=True)
                  vt = vpool.tile([P, 97 * H], MF, name=f"v{st}", tag=f"v{st}")
                  nc.scalar.copy(vt[:], ps[:])
                  v_sb.append(vt)

              # ---- attention ----
              for ci in range(NCH):
                  on_tiles = []
                  for h in range(H):
                      nkt = 4 * (ci + 1)
                      acc = accpool.tile([P, CH], F32, name="acc", tag="acc")
                      kt0 = 0
                      for gsize in _split_groups(nkt, GRP):
                          kts = list(range(kt0, kt0 + gsize))
                          kt0 += gsize
                          qk = qkpool.tile([P, GRP * CH], F32, name="qk", tag="qk")
                          for j, kt in enumerate(kts):
                              nc.tensor.matmul(
                                  qk[:, CH * j:CH * (j + 1)],
                                  kt_sb[h][:, P * kt:P * kt + P],
                                  qt_sb[h][:, CH * ci:CH * ci + CH],
                                  start=True, stop=True)
                          pt = ppool.tile([P, GRP * CH], MF, name="pt", tag="pt")
                          L = CH * gsize
                          nc.scalar.activation(pt[:, :L], qk[:, :L], Exp, scale=float(SCALE))
                          for j, kt in enumerate(kts):
                              rt = P * kt - CH * ci
                              if rt >= 0:
                                  # zero the upper triangle of the 128x128
                                  # diagonal block; cols below rt are skipped
                                  # by the PV matmul
                                  nc.vector.tensor_mul(
                                      pt[:, CH * j + rt:CH * j + rt + P],
                                      pt[:, CH * j + rt:CH * j + rt + P],
                                      msk_sb[:, 0:P])
                          for j, kt in enumerate(kts):
                              rt = P * kt - CH * ci
                              scol = max(rt, 0)
                              nc.tensor.matmul(
                                  acc[0:97, scol:CH],
                                  v_sb[kt][:, 97 * h:97 * h + 97],
                                  pt[:, CH * j + scol:CH * (j + 1)],
                                  start=(kt == 0), stop=(kt == nkt - 1),
                                  skip_group_check=True)
                      # normalize: row 96 of acc is the softmax denominator.
                      # custom-DVE ops and partition_broadcast only address
                      # partition 0, so stage the row there via a small DMA.
                      den96 = rpool.tile([97, CH], F32, name="den96", tag="den96")
                      nc.vector.tensor_copy(den96[96:97, :], acc[96:97, :])
                      den0 = rpool.tile([1, CH], F32, name="den0", tag="den0")
                      nc.sync.dma_start(den0[:], den96[96:97, :])
                      nc.vector.reciprocal_approx_fast(out=den0[:], in_=den0[:])
                      rb = rpool.tile([HD, CH], F32, name="rb", tag="rb")
                      nc.gpsimd.partition_broadcast(rb[:], den0[:], channels=HD)
                      on = onpool.tile([HD, CH], MF, name=f"on{h}", tag=f"on{h}")
                      nc.vector.tensor_tensor(on[:], acc[0:HD, :], rb[:], op=mult)
                      on_tiles.append(on)
                  # output projection for this chunk's 4 row-tiles
                  for sj in range(4):
                      st = 4 * ci + sj
                      fo = accpool.tile([P, D], F32, name="fo", tag="acc")
                      for h in range(H):
                          nc.tensor.matmul(fo[:], on_tiles[h][:, P * sj:P * sj + P],
                                           wo_sb[h][:], start=(h == 0), stop=False)
                      nc.tensor.matmul(fo[:], ones[:, 0:P], bo_sb[:],
                                       start=False, stop=True)
                      fs = onpool.tile([P, D], F32, name="fs", tag="fs", bufs=3)
                      nc.scalar.copy(fs[:], fo[:])
                      nc.sync.dma_start(out_d[P * st:P * st + P, :], fs[:])

        for pool in (accpool, qkpool, rpool, onpool, ppool, vpool,
                     qkt_pool, xpool, wpool):
            pool.release()

    nc.finalize()
    return nc


_NC_CACHE = None


def get_nc():
    global _NC_CACHE
    if _NC_CACHE is None:
        _NC_CACHE = build_nc()
    return _NC_CACHE


def host_prep(x, Wq, bq, Wk, bk, Wv, bv, Wo, bo):
    """Build per-core input maps (layout prep only; all FLOPs run on device)."""
    x = np.ascontiguousarray(np.asarray(x, dtype=np.float32))
    Wq = np.ascontiguousarray(np.asarray(Wq, dtype=np.float32))
    Wk = np.ascontiguousarray(np.asarray(Wk, dtype=np.float32))
    Wv = np.ascontiguousarray(np.asarray(Wv, dtype=np.float32))
    Wo = np.ascontiguousarray(np.asarray(Wo, dtype=np.float32))
    bq = np.asarray(bq, dtype=np.float32)
    bk = np.asarray(bk, dtype=np.float32)
    bv = np.asarray(bv, dtype=np.float32)
    bo = np.asarray(bo, dtype=np.float32)

    wvx = np.zeros((D + 1, 97 * H), np.float32)
    for h in range(H):
        wvx[:D, 97 * h:97 * h + HD] = Wv[:, HD * h:HD * h + HD]
        wvx[D, 97 * h:97 * h + HD] = bv[HD * h:HD * h + HD]
        wvx[D, 97 * h + HD] = 1.0

    jj = np.arange(CH)[None, :]
    pp = np.arange(P)[:, None]
    msk = np.zeros((P, 4 * CH), np.float32)
    for r in range(4):
        msk[:, CH * r:CH * r + CH] = (jj >= P * r + pp).astype(np.float32)

    bqh = np.ascontiguousarray(bq.reshape(H, HD).T)
    bkh = np.ascontiguousarray(bk.reshape(H, HD).T)
    common = dict(wq=Wq, wk=Wk, wvx=wvx, wo=Wo, bqh=bqh, bkh=bkh,
                  bo=np.ascontiguousarray(bo.reshape(1, D)), msk=msk,
                  onesrow=np.ones((1, S), np.float32))
    return [dict(xt=np.ascontiguousarray(x[b].T), **common)
            for b in range(x.shape[0])]


def kernel(**inputs):
    in_maps = host_prep(**inputs)
    nc = get_nc()
    res = run_bass_kernel_spmd(nc, in_maps, core_ids=list(range(N_CORES)))
    return np.stack([res.results[b]["out"] for b in range(N_CORES)], axis=0)



# revision 6
# speedup vs baseline: 2.4114x; 2.4114x over previous
"""Causal multi-head self-attention (B=8, S=2048, D=384, H=4, Hd=96) on 8
Trainium2 NeuronCores.

Sharding: data-parallel over batch — each core processes one batch element,
weights replicated. No collectives needed.

Per-core algorithm (flash-style, fully SBUF-resident; all matmul inputs
bf16, PSUM accumulation fp32):
  - host passes x[b] pre-transposed as xt [384, 2048] bf16
  - QT/KT computed per head, per 512-col chunk, in [96, S] layout; the
    PSUM->SBUF move is a DVE tensor_scalar that folds in the bias
  - V' natural layout [S, 97*4]: per head [ones_col | V_h]; the ones column
    and bias come from a broadcast [128, 388] tile added on the DVE during
    the PSUM->SBUF move
  - scoresT[k, q] = KT_h^T @ QT_h per 128-row k-tile and 512-col q-chunk,
    left-trimmed to the causal hull; exp on ScalarE (scale=1/sqrt(Hd)
    folded); diagonal 128x128 blocks masked post-exp on the DVE (bf16 2x)
  - PV into PSUM acc [97, 512]: row 0 = softmax denominator (ones-first
    V'), rows 1..96 = unnormalized head output; reciprocal directly from
    PSUM on the DVE custom op, partition_broadcast on GpSimd, normalize on
    DVE (bf16 out)
  - out projection per 128-row tile: 4 matmuls (lhsT = normalized tiles at
    partition offset 1), bias added by the DVE during the PSUM->SBUF move,
    DMA to HBM
  - emission is software-pipelined: scores/exp of group g+1 are emitted
    before PV of group g (pending-FIFO), per-chunk projections and the
    previous chunk's out-projection backfill PE gaps
"""

import sys

sys.path.insert(0, "/opt/trn_rl_repo")

import ml_dtypes
import numpy as np

import concourse.bass as bass
import concourse.tile as tile
from concourse import bacc, mybir
from concourse.bass_utils import run_bass_kernel_spmd

N_CORES = 8
S = 2048
D = 384
H = 4
HD = 96
CH = 512          # q-chunk width
NCH = S // CH     # 4 q-chunks
P = 128           # k-tile height / partition dim
KTN = S // P      # 16 k-tiles
SCALE = 1.0 / np.sqrt(HD)
VW = 128          # V' cols per head: [ones | 31*0 | V_h] (padded so
VOFF = 32         # out-proj lhsT can start at base partition 32)

F32 = mybir.dt.float32
BF16 = mybir.dt.bfloat16


def build_nc(repeat=1, variant=(), loop_n=0):
    nc = bacc.Bacc("TRN2", target_bir_lowering=False, debug=False,
                   enable_asserts=False, num_devices=N_CORES)

    xt_d = nc.dram_tensor("xt", [D, S], BF16, kind="ExternalInput").ap()
    wq_d = nc.dram_tensor("wq", [D, D], BF16, kind="ExternalInput").ap()
    wk_d = nc.dram_tensor("wk", [D, D], BF16, kind="ExternalInput").ap()
    wv_d = nc.dram_tensor("wv", [D, VW * H], BF16, kind="ExternalInput").ap()
    wo_d = nc.dram_tensor("wo", [VW * H, D], BF16, kind="ExternalInput").ap()
    bqh_d = nc.dram_tensor("bqh", [HD, H], F32, kind="ExternalInput").ap()
    bkh_d = nc.dram_tensor("bkh", [HD, H], F32, kind="ExternalInput").ap()
    vb_d = nc.dram_tensor("vb", [1, VW * H], F32, kind="ExternalInput").ap()
    bo_d = nc.dram_tensor("bo", [1, D], F32, kind="ExternalInput").ap()
    msk_d = nc.dram_tensor("msk", [P, P], BF16, kind="ExternalInput").ap()
    out_d = nc.dram_tensor("out", [S, D], F32, kind="ExternalOutput").ap()

    Exp = mybir.ActivationFunctionType.Exp
    mult = mybir.AluOpType.mult
    add = mybir.AluOpType.add

    with tile.TileContext(nc) as tc:
        wpool = tc.alloc_tile_pool(name="w", bufs=1)
        xpool = tc.alloc_tile_pool(name="x", bufs=1)
        qkt_pool = tc.alloc_tile_pool(name="qkt", bufs=1)
        vpool = tc.alloc_tile_pool(name="v", bufs=1)
        ppool = tc.alloc_tile_pool(name="p", bufs=3)
        onpool = tc.alloc_tile_pool(name="on", bufs=1)
        rpool = tc.alloc_tile_pool(name="r", bufs=3)
        fspool = tc.alloc_tile_pool(name="fs", bufs=3)
        qkpool = tc.alloc_tile_pool(name="qkps", bufs=2, space="PSUM")
        accpool = tc.alloc_tile_pool(name="accps", bufs=2, space="PSUM")
        pspool = tc.alloc_tile_pool(name="pps", bufs=2, space="PSUM")

        import contextlib
        loop_ctx = (tc.For_i(0, loop_n, 1) if loop_n
                    else contextlib.nullcontext())
        with loop_ctx:
          for _rep in range(repeat):
            # ---- load constants / weights (small tensors first so the
            # act-table warmup + bias broadcasts run during the big loads) ----
            msk_sb = wpool.tile([P, P], BF16, name="msk", tag="msk")
            nc.sync.dma_start(msk_sb[:], msk_d[:, :])
            bq_sb = wpool.tile([HD, H], F32, name="bq", tag="bq")
            nc.sync.dma_start(bq_sb[:], bqh_d[:, :])
            bk_sb = wpool.tile([HD, H], F32, name="bk", tag="bk")
            nc.sync.dma_start(bk_sb[:], bkh_d[:, :])
            vb_sb = wpool.tile([1, VW * H], F32, name="vb", tag="vb")
            nc.sync.dma_start(vb_sb[:], vb_d[:, :])
            bo_sb = wpool.tile([1, D], F32, name="bo", tag="bo")
            nc.sync.dma_start(bo_sb[:], bo_d[:, :])

            # warmup: force the Exp act-table load off the critical path
            warm = wpool.tile([1, 1], BF16, name="warm", tag="warm")
            nc.scalar.activation(warm[:], msk_sb[0:1, 0:1], Exp)

            # broadcast V'/out biases across partitions once (GpSimd)
            vb_bc = wpool.tile([P, VW * H], F32, name="vbbc", tag="vbbc")
            nc.gpsimd.partition_broadcast(vb_bc[:], vb_sb[:], channels=P)
            bo_bc = wpool.tile([P, D], F32, name="bobc", tag="bobc")
            nc.gpsimd.partition_broadcast(bo_bc[:], bo_sb[:], channels=P)

            xt_sb, wq_sb, wk_sb, wv_sb, wo_sb = [], [], [], [], []
            for t in range(3):
                xt = xpool.tile([P, S], BF16, name=f"xt{t}", tag=f"xt{t}")
                nc.sync.dma_start(xt[:], xt_d[P * t:P * t + P, :])
                xt_sb.append(xt)
                wqt = wpool.tile([P, D], BF16, name=f"wq{t}", tag=f"wq{t}")
                nc.sync.dma_start(wqt[:], wq_d[P * t:P * t + P, :])
                wq_sb.append(wqt)
                wkt = wpool.tile([P, D], BF16, name=f"wk{t}", tag=f"wk{t}")
                nc.sync.dma_start(wkt[:], wk_d[P * t:P * t + P, :])
                wk_sb.append(wkt)
                wvt = wpool.tile([P, VW * H], BF16, name=f"wv{t}", tag=f"wv{t}")
                nc.sync.dma_start(wvt[:], wv_d[P * t:P * t + P, :])
                wv_sb.append(wvt)
            for h in range(H):
                # rows 0..31 are zeros (host-padded) so the full-128
                # contraction in the out projection ignores the
                # denominator/pad rows of the on tiles
                wot = wpool.tile([P, D], BF16, name=f"wo{h}", tag=f"wo{h}")
                nc.sync.dma_start(wot[:], wo_d[VW * h:VW * h + P, :])
                wo_sb.append(wot)

            # persistent SBUF results
            qt_sb = [qkt_pool.tile([HD, S], BF16, name=f"qt{h}", tag=f"qt{h}")
                     for h in range(H)]
            kt_sb = [qkt_pool.tile([HD, S], BF16, name=f"kt{h}", tag=f"kt{h}")
                     for h in range(H)]
            v_sb = [vpool.tile([P, VW * H], BF16, name=f"v{st}", tag=f"v{st}")
                    for st in range(KTN)]
            on_sb = {}
            for ci in range(NCH):
                for h in range(H):
                    on_sb[(ci, h)] = onpool.tile(
                        [VW, CH], BF16, name=f"on{ci}_{h}",
                        tag=f"on{ci}_{h}")

            pending = []

            def pump():
                if pending:
                    pending.pop(0)()

            def emit_qk_proj(h, ci):
                for w_sb, b_sb, dst in ((wq_sb, bq_sb, qt_sb),
                                        (wk_sb, bk_sb, kt_sb)):
                    ps = pspool.tile([HD, CH], F32, name="projps", tag="pps")
                    for t in range(3):
                        nc.tensor.matmul(
                            ps[:],
                            w_sb[t][:, HD * h:HD * h + HD],
                            xt_sb[t][:, CH * ci:CH * ci + CH],
                            start=(t == 0), stop=(t == 2))
                    nc.vector.tensor_scalar(
                        dst[h][:, CH * ci:CH * ci + CH], ps[:],
                        b_sb[:, h:h + 1], None, add)

            def emit_v_proj(st):
                ps = pspool.tile([P, VW * H], F32, name="vps", tag="pps")
                for t in range(3):
                    nc.tensor.matmul(ps[:], xt_sb[t][:, P * st:P * st + P],
                                     wv_sb[t][:], start=(t == 0), stop=(t == 2))
                nc.vector.tensor_tensor(v_sb[st][:], ps[:], vb_bc[:], op=add)

            def emit_out_proj(ci, sj):
                st = 4 * ci + sj
                fo = pspool.tile([P, D], F32, name="fo", tag="pps")
                for h in range(H):
                    nc.tensor.matmul(
                        fo[:], on_sb[(ci, h)][:, P * sj:P * sj + P],
                        wo_sb[h][:], start=(h == 0), stop=(h == 3))
                fs = fspool.tile([P, D], F32, name="fs", tag="fs")
                nc.vector.tensor_tensor(fs[:], fo[:], bo_bc[:], op=add)
                nc.sync.dma_start(out_d[P * st:P * st + P, :], fs[:])

            def make_pv(ci, h, kts, qk_rts, pt, nkt):
                acc = pv_acc[(ci, h)]

                def pv():
                    for j, kt in enumerate(kts):
                        rt = P * kt - CH * ci
                        scol = max(rt, 0)
                        nc.tensor.matmul(
                            acc[0:VW, scol:CH],
                            v_sb[kt][:, VW * h:VW * h + VW],
                            pt[:, CH * j + scol:CH * (j + 1)],
                            start=(kt == 0), stop=(kt == nkt - 1),
                            skip_group_check=True)
                    if kts[-1] == nkt - 1:
                        # chunk-head finished: normalize
                        den0 = rpool.tile([1, CH], F32, name="den0", tag="den0")
                        nc.vector.reciprocal_approx_fast(
                            out=den0[:], in_=acc[0:1, :])
                        rb = rpool.tile([VW, CH], F32, name="rb", tag="rb")
                        nc.gpsimd.partition_broadcast(rb[:], den0[:],
                                                      channels=VW)
                        nc.vector.tensor_tensor(
                            on_sb[(ci, h)][:], acc[0:VW, :], rb[:], op=mult)
                        if h == H - 1:
                            for sj in range(4):
                                pending.append(
                                    lambda ci=ci, sj=sj: emit_out_proj(ci, sj))
                return pv

            pv_acc = {}
            for ci in range(NCH):
                for h in range(H):
                    emit_qk_proj(h, ci)
                for sj in range(4):
                    emit_v_proj(4 * ci + sj)
                for h in range(H):
                    nkt = 4 * (ci + 1)
                    acc = accpool.tile([VW, CH], F32, name="acc", tag="acc")
                    pv_acc[(ci, h)] = acc
                    for g0 in range(0, nkt, 2):
                        kts = [g0, g0 + 1]
                        qk = qkpool.tile([P, 2 * CH], F32, name="qk", tag="qk")
                        pt = ppool.tile([P, 2 * CH], BF16, name="pt", tag="pt")
                        rts = []
                        for j, kt in enumerate(kts):
                            rt = P * kt - CH * ci
                            rts.append(rt)
                            scol = max(rt, 0)
                            nc.tensor.matmul(
                                qk[:, CH * j + scol:CH * (j + 1)],
                                kt_sb[h][:, P * kt:P * kt + P],
                                qt_sb[h][:, CH * ci + scol:CH * ci + CH],
                                start=True, stop=True)
                        if rts[0] >= 0:
                            # diagonal pair: per-tile trimmed exp + mask
                            for j, rt in enumerate(rts):
                                nc.scalar.activation(
                                    pt[:, CH * j + rt:CH * (j + 1)],
                                    qk[:, CH * j + rt:CH * (j + 1)],
                                    Exp, scale=float(SCALE))
                                nc.vector.tensor_tensor(
                                    pt[:, CH * j + rt:CH * j + rt + P],
                                    pt[:, CH * j + rt:CH * j + rt + P],
                                    msk_sb[:], op=mult)
                        else:
                            nc.scalar.activation(
                                pt[:], qk[:], Exp, scale=float(SCALE))
                        pump()
                        pending.append(make_pv(ci, h, kts, rts, pt, nkt))
            while pending:
                pump()

        for pool in (pspool, accpool, qkpool, fspool, rpool, onpool, ppool,
                     vpool, qkt_pool, xpool, wpool):
            pool.release()

    nc.finalize()
    return nc


_NC_CACHE = None


def get_nc():
    global _NC_CACHE
    if _NC_CACHE is None:
        _NC_CACHE = build_nc()
    return _NC_CACHE


def host_prep(x, Wq, bq, Wk, bk, Wv, bv, Wo, bo):
    """Build per-core input maps (layout/dtype prep only)."""
    bf = ml_dtypes.bfloat16
    x = np.asarray(x, dtype=np.float32)
    Wq = np.asarray(Wq, dtype=np.float32).astype(bf)
    Wk = np.asarray(Wk, dtype=np.float32).astype(bf)
    Wv = np.asarray(Wv, dtype=np.float32)
    Wo = np.asarray(Wo, dtype=np.float32)
    bq = np.asarray(bq, dtype=np.float32)
    bk = np.asarray(bk, dtype=np.float32)
    bv = np.asarray(bv, dtype=np.float32)
    bo = np.asarray(bo, dtype=np.float32)

    wo_pad = np.zeros((VW * H, D), np.float32)
    for h in range(H):
        wo_pad[VW * h + VOFF:VW * h + VW] = Wo[HD * h:HD * h + HD]

    wv_x = np.zeros((D, VW * H), np.float32)
    vbias = np.zeros((1, VW * H), np.float32)
    for h in range(H):
        wv_x[:, VW * h + VOFF:VW * h + VW] = Wv[:, HD * h:HD * h + HD]
        vbias[0, VW * h] = 1.0
        vbias[0, VW * h + VOFF:VW * h + VW] = bv[HD * h:HD * h + HD]

    jj = np.arange(P)[None, :]
    pp = np.arange(P)[:, None]
    msk = (jj >= pp).astype(bf)

    bqh = np.ascontiguousarray(bq.reshape(H, HD).T)
    bkh = np.ascontiguousarray(bk.reshape(H, HD).T)
    common = dict(wq=np.ascontiguousarray(Wq), wk=np.ascontiguousarray(Wk),
                  wv=wv_x.astype(bf), wo=wo_pad.astype(bf),
                  bqh=bqh, bkh=bkh, vb=vbias,
                  bo=np.ascontiguousarray(bo.reshape(1, D)), msk=msk)
    return [dict(xt=np.ascontiguousarray(x[b].T.astype(bf)), **common)
            for b in range(x.shape[0])]


def kernel(**inputs):
    in_maps = host_prep(**inputs)
    nc = get_nc()
    res = run_bass_kernel_spmd(nc, in_maps, core_ids=list(range(N_CORES)))
    return np.stack([res.results[b]["out"] for b in range(N_CORES)], axis=0)
